# revision 6
# baseline (speedup 1.0000x reference)
"""Multi-head GAT layer (PyG GATConv semantics + skip + ELU) on 8 Trainium2 NeuronCores.

Strategy (dst-sharded message passing):
  - Pad nodes to N_pad = 8*BPC*128. Core c owns nodes [c*BPC*128, (c+1)*BPC*128).
  - Phase 1 (per core): h_aug = x @ [W | A_src | A_dst | skip_W] for the core's own
    node shard on the TensorEngine (fp32r). Rows [h | a_src] (260 f32) go to a DRAM
    shard table; the full h_aug stays in SBUF (skip path, a_dst, self-loop terms).
  - AllGather the 260-col table -> full table (Shared DRAM).
  - Phase 2: real edges (self-loops handled separately) sorted by dst, bucketed per
    128-node dst block (host side). Per 128-edge tile:
      * indirect-DMA gather of hplus[src] rows -> G [128, 260]  (1 op per tile; the
        SWDGE Q7 emission of ~1us/op is the kernel's critical path)
      * a_dst node->edge expansion: T = PE-transpose(rel bcast), ST = is_equal(T,
        iota) [128n x 128e], e4 = ST.T @ a_dst_blk (tiny matmul)
      * e = a_src + e4; leakyrelu; exp (shift-free softmax: logits are small and
        every node has a self-loop, so no max subtraction / no overflow)
      * msg = exp (x) h; scatter-add U += S.T @ [msg | exp] with one-hot
        S[e,n] = (rel[e]==n); padding edges carry rel=255 -> contribute nothing.
  - Per-block epilogue: add self-loop term exp_self*h_own locally, divide by
    denom, add skip (+bias), ELU, DMA out.

Per-(local block) tile counts are max'ed over the 8 cores so one SPMD program
fits all cores; excess tiles are padding.
"""

import numpy as np

from concourse import bass, bacc, mybir, tile
from concourse.bass_utils import run_bass_kernel_spmd
from concourse.masks import make_identity

P = 128
HEADS = 4
C = 64
HC = HEADS * C          # 256
IN_DIM = 256
GROW = HC + HEADS       # 260 gather-row cols: h | a_src
HAUG = GROW + HEADS     # 264: + a_dst
WCOLS = HAUG + HC       # 520: + skip
NCORES = 8
NEG_SLOPE = 0.2
EPS = 1e-16
KG = 8                  # tiles per transform group

F32 = mybir.dt.float32
F32R = mybir.dt.float32r
I32 = mybir.dt.int32
AF = mybir.ActivationFunctionType
OP = mybir.AluOpType


# ----------------------------------------------------------------------------- host prep

def _plan(edge_index: np.ndarray, n_real: int):
    bpc = int(np.ceil(n_real / (NCORES * P)))
    n_pad = NCORES * bpc * P
    nblk = NCORES * bpc

    src = np.ascontiguousarray(edge_index[0]).astype(np.int64)
    dst = np.ascontiguousarray(edge_index[1]).astype(np.int64)

    order = np.argsort(dst, kind="stable")
    s_sorted = src[order].astype(np.int32)
    d_sorted = dst[order].astype(np.int32)

    blk_of_edge = d_sorted >> 7
    counts = np.bincount(blk_of_edge, minlength=nblk)
    starts = np.concatenate([[0], np.cumsum(counts)])

    cnt_cb = counts.reshape(NCORES, bpc)
    tb = np.maximum(1, np.ceil(cnt_cb / P).astype(np.int64)).max(axis=0)
    t_total = int(tb.sum())
    tstart = np.concatenate([[0], np.cumsum(tb)])

    src_T = np.zeros((NCORES, P, t_total), dtype=np.int32)
    rel_T = np.full((NCORES, P, t_total), 255.0, dtype=np.float32)

    for c in range(NCORES):
        for b in range(bpc):
            g = c * bpc + b
            e0, e1 = starts[g], starts[g + 1]
            cnt = e1 - e0
            ntile = int(tb[b])
            cap = ntile * P
            bs = np.zeros(cap, dtype=np.int32)
            br = np.full(cap, 255.0, dtype=np.float32)
            bs[:cnt] = s_sorted[e0:e1]
            br[:cnt] = (d_sorted[e0:e1] - g * P).astype(np.float32)
            t0 = tstart[b]
            src_T[c, :, t0:t0 + ntile] = bs.reshape(ntile, P).T
            rel_T[c, :, t0:t0 + ntile] = br.reshape(ntile, P).T

    return dict(bpc=bpc, n_pad=n_pad, tb=tb.tolist(), t_total=t_total,
                tstart=tstart.tolist(), src_T=src_T, rel_T=rel_T)


def _weights(W, att_src, att_dst, bias, skip_W):
    has_bias = bool(np.any(bias != 0.0))
    kdim = IN_DIM + (1 if has_bias else 0)
    w_all = np.zeros((kdim, WCOLS), dtype=np.float32)
    w_all[:IN_DIM, 0:HC] = W
    wr = W.reshape(IN_DIM, HEADS, C)
    w_all[:IN_DIM, HC:HC + HEADS] = np.einsum("khc,hc->kh", wr, att_src)
    w_all[:IN_DIM, GROW:HAUG] = np.einsum("khc,hc->kh", wr, att_dst)
    w_all[:IN_DIM, HAUG:] = skip_W
    if has_bias:
        w_all[IN_DIM, HAUG:] = bias
    return w_all, has_bias


# ----------------------------------------------------------------------------- program

def _b(ap, n):
    """Broadcast: append 0-stride dim of size n."""
    ap = ap.unsqueeze(ap.ndim)
    return ap.to_broadcast(list(ap.shape[:-1]) + [n])


def build_program(bpc, t_total, tb, tstart, has_bias):
    kdim = IN_DIM + (1 if has_bias else 0)
    rows_c = bpc * P
    nc = bacc.Bacc("TRN2", target_bir_lowering=False, debug=False,
                   num_devices=NCORES)

    xT_blocks = nc.dram_tensor("xT_blocks", [bpc, kdim, P], F32R, kind="ExternalInput").ap()
    w_all = nc.dram_tensor("w_all", [kdim, WCOLS], F32R, kind="ExternalInput").ap()
    src_T = nc.dram_tensor("src_T", [P, t_total], I32, kind="ExternalInput").ap()
    rel_T = nc.dram_tensor("rel_T", [P, t_total], F32, kind="ExternalInput").ap()
    out_shard = nc.dram_tensor("out_shard", [rows_c, HC], F32, kind="ExternalOutput").ap()

    replica = [list(range(NCORES))]
    blk_of_tile = np.searchsorted(np.asarray(tstart), np.arange(t_total), side="right") - 1

    with tile.TileContext(nc) as tc:
        with (
            tc.tile_pool(name="dram", bufs=1, space="DRAM") as dram,
            tc.tile_pool(name="persist", bufs=1) as persist,
            tc.tile_pool(name="xio", bufs=3) as xio,
            tc.tile_pool(name="gio", bufs=4) as gio,
            tc.tile_pool(name="sio", bufs=3) as sio,
            tc.tile_pool(name="stio", bufs=4) as stio,
            tc.tile_pool(name="epi", bufs=2) as epi,
            tc.tile_pool(name="psum_h", bufs=1, space="PSUM") as psum_h,
            tc.tile_pool(name="psum_u", bufs=2, space="PSUM") as psum_u,
            tc.tile_pool(name="psum_t", bufs=1, space="PSUM") as psum_t,
            tc.tile_pool(name="psum_e", bufs=2, space="PSUM") as psum_e,
        ):
            hplus_shard = dram.tile([rows_c, GROW], F32R)
            hplus_full = dram.tile([NCORES * rows_c, GROW], F32R, addr_space="Shared")

            # ---- persistent SBUF
            h_all = persist.tile([P, bpc * WCOLS], F32R)   # per block: h|a_src|a_dst|skip
            src_sb = persist.tile([P, t_total], I32)
            rel_sb = persist.tile([P, t_total], F32)
            iota_sb = persist.tile([P, P], F32)
            iota_col = persist.tile([P, 1], F32)
            ident = persist.tile([P, P], F32)
            iota_i = persist.tile([P, P], I32)
            iota_ci = persist.tile([P, 1], I32)
            nc.sync.dma_start(out=src_sb[:], in_=src_T[:, :])
            nc.sync.dma_start(out=rel_sb[:], in_=rel_T[:, :])
            nc.gpsimd.iota(iota_i[:], pattern=[[1, P]], base=0, channel_multiplier=0)
            nc.gpsimd.iota(iota_ci[:], pattern=[[0, 1]], base=0, channel_multiplier=1)
            nc.vector.tensor_copy(iota_sb[:], iota_i[:])
            nc.vector.tensor_copy(iota_col[:], iota_ci[:])
            make_identity(nc, ident[:])

            kchunks = [(0, 128), (128, 128)] + ([(256, 1)] if has_bias else [])
            w_sb = []
            for i, (k0, kn) in enumerate(kchunks):
                wt = persist.tile([kn, WCOLS], F32R, name=f"w_sb{i}")
                nc.sync.dma_start(out=wt[:], in_=w_all[k0:k0 + kn, :])
                w_sb.append(wt)

            # ---- phase 1: h_aug for own shard
            for b in range(bpc):
                xts = []
                for i, (k0, kn) in enumerate(kchunks):
                    xt = xio.tile([kn, P], F32R, tag=f"xt{i}", name=f"xt{i}_{b}")
                    nc.sync.dma_start(out=xt[:], in_=xT_blocks[b, k0:k0 + kn, :])
                    xts.append(xt)
                hA = psum_h.tile([P, HAUG], F32, tag="hA", space="PSUM", name=f"hA{b}")
                hB = psum_h.tile([P, HC], F32, tag="hB", space="PSUM", name=f"hB{b}")
                for i in range(len(kchunks)):
                    nc.tensor.matmul(hA[:], xts[i][:], w_sb[i][:, 0:HAUG],
                                     start=(i == 0), stop=(i == len(kchunks) - 1))
                for i in range(len(kchunks)):
                    nc.tensor.matmul(hB[:], xts[i][:], w_sb[i][:, HAUG:WCOLS],
                                     start=(i == 0), stop=(i == len(kchunks) - 1))
                hs = h_all[:, b * WCOLS:b * WCOLS + HAUG]
                nc.vector.tensor_copy(hs, hA[:])
                nc.vector.tensor_copy(h_all[:, b * WCOLS + HAUG:(b + 1) * WCOLS], hB[:])
                nc.sync.dma_start(out=hplus_shard[b * P:(b + 1) * P, :],
                                  in_=h_all[:, b * WCOLS:b * WCOLS + GROW])

            # ---- all-gather
            nc.gpsimd.collective_compute(
                "AllGather", mybir.AluOpType.bypass, replica_groups=replica,
                ins=[hplus_shard.opt()], outs=[hplus_full.opt()],
            )

            # ---- phase 2
            n_groups = (t_total + KG - 1) // KG
            u_psum = {}
            for g in range(n_groups):
                t0 = g * KG
                k = min(KG, t_total - t0)
                gbuf = gio.tile([P, KG * GROW], F32R, tag="gbuf", name=f"gbuf{g}")
                e4g = psum_e.tile([P, KG * HEADS], F32, tag="e4g", space="PSUM",
                                  name=f"e4g{g}")
                tps = psum_t.tile([P, KG * P], F32, tag="tps", space="PSUM",
                                  name=f"tps{g}")
                stg = stio.tile([P, KG * P], F32R, tag="stg", name=f"stg{g}")
                for j in range(k):
                    t = t0 + j
                    nc.gpsimd.indirect_dma_start(
                        out=gbuf[:, j * GROW:(j + 1) * GROW], out_offset=None,
                        in_=hplus_full[:, :],
                        in_offset=bass.IndirectOffsetOnAxis(ap=src_sb[:, t:t + 1], axis=0),
                    )
                    # T[n,e] = rel[e]; ST = (T == iota_col)
                    nc.tensor.transpose(
                        out=tps[:, j * P:(j + 1) * P],
                        in_=rel_sb[:, t:t + 1].to_broadcast([P, P]),
                        identity=ident[:])
                # ST build + a_dst expansion
                t3 = tps[:].rearrange("p (t n) -> p t n", n=P)[:, 0:k, :]
                st3 = stg[:].rearrange("p (t n) -> p t n", n=P)[:, 0:k, :]
                nc.vector.tensor_tensor(out=st3, in0=t3, in1=iota_col[:, 0:1].unsqueeze(1).to_broadcast([P, k, P]),
                                        op=OP.is_equal)
                for j in range(k):
                    t = t0 + j
                    b = int(blk_of_tile[t])
                    nc.tensor.matmul(
                        e4g[:, j * HEADS:(j + 1) * HEADS],
                        lhsT=stg[:, j * P:(j + 1) * P],
                        rhs=h_all[:, b * WCOLS + GROW:b * WCOLS + HAUG],
                        start=True, stop=True)
                # e = a_src + a_dst ; leaky ; exp  (batched over the group)
                g3 = gbuf[:].rearrange("p (t f) -> p t f", f=GROW)
                e_sl = g3[:, 0:k, HC:GROW]
                e4s = e4g[:].rearrange("p (t f) -> p t f", f=HEADS)[:, 0:k, :]
                nc.vector.tensor_tensor(out=e_sl, in0=e_sl, in1=e4s, op=OP.add)
                nc.vector.scalar_tensor_tensor(out=e_sl, in0=e_sl, scalar=NEG_SLOPE,
                                               in1=e_sl, op0=OP.mult, op1=OP.max)
                nc.scalar.activation(e_sl, e_sl, AF.Exp)
                # msg = h * exp
                m4 = g3[:, 0:k, 0:HC].rearrange("p t (h c) -> p t h c", c=C)
                nc.vector.tensor_tensor(out=m4, in0=m4, in1=_b(g3[:, 0:k, HC:GROW], C),
                                        op=OP.mult)
                # one-hot S for the whole group
                sbuf_t = sio.tile([P, KG * P], F32R, tag="sbuf_t", name=f"sbt{g}")
                s3 = sbuf_t[:].rearrange("p (t n) -> p t n", n=P)[:, 0:k, :]
                nc.vector.tensor_tensor(out=s3, in0=_b(rel_sb[:, t0:t0 + k], P),
                                        in1=iota_sb[:].unsqueeze(1).to_broadcast([P, k, P]),
                                        op=OP.is_equal)
                for j in range(k):
                    t = t0 + j
                    b = int(blk_of_tile[t])
                    first = (t == tstart[b])
                    last = (t == tstart[b + 1] - 1)
                    if first:
                        u_psum[b] = psum_u.tile([P, GROW], F32, tag="u_psum",
                                                space="PSUM", name=f"u{b}")
                    nc.tensor.matmul(
                        u_psum[b][:],
                        lhsT=sbuf_t[:, j * P:(j + 1) * P],
                        rhs=gbuf[:, j * GROW:(j + 1) * GROW],
                        start=first, stop=last)
                    if last:
                        _epilogue(nc, epi, u_psum.pop(b), h_all, out_shard, b)

    nc.compile()
    return nc


def _epilogue(nc, epi, U, h_all, out_shard, b):
    w0 = b * WCOLS
    # self-loop: e_self = a_src_own + a_dst_own -> leaky -> exp
    es = epi.tile([P, HEADS], F32, tag="es", name=f"es{b}")
    nc.vector.tensor_tensor(out=es[:], in0=h_all[:, w0 + HC:w0 + GROW].bitcast(F32),
                            in1=h_all[:, w0 + GROW:w0 + HAUG].bitcast(F32), op=OP.add)
    nc.vector.scalar_tensor_tensor(out=es[:], in0=es[:], scalar=NEG_SLOPE,
                                   in1=es[:], op0=OP.mult, op1=OP.max)
    nc.scalar.activation(es[:], es[:], AF.Exp)
    # denom = U[:,256:260] + exp_self + eps ; r = 1/denom
    r4 = epi.tile([P, HEADS], F32, tag="r4", name=f"r4{b}")
    nc.vector.scalar_tensor_tensor(out=r4[:], in0=U[:, HC:GROW], scalar=EPS,
                                   in1=es[:], op0=OP.add, op1=OP.add)
    nc.vector.reciprocal(r4[:], r4[:])
    # pre = (U + h_own*exp_self) * r + skip
    pre = epi.tile([P, HC], F32, tag="pre", name=f"pre{b}")
    p4 = pre[:].rearrange("p (h c) -> p h c", c=C)
    h4 = h_all[:, w0:w0 + HC].bitcast(F32).rearrange("p (h c) -> p h c", c=C)
    nc.vector.tensor_tensor(out=p4, in0=h4, in1=_b(es[:], C), op=OP.mult)
    nc.vector.tensor_tensor(out=pre[:], in0=pre[:], in1=U[:, 0:HC], op=OP.add)
    nc.vector.tensor_tensor(out=p4, in0=p4, in1=_b(r4[:], C), op=OP.mult)
    nc.vector.tensor_tensor(out=pre[:], in0=pre[:],
                            in1=h_all[:, w0 + HAUG:w0 + WCOLS].bitcast(F32), op=OP.add)
    # ELU(x) = max(x, exp(min(x,0)) - 1)
    m = epi.tile([P, HC], F32, tag="m", name=f"m{b}")
    nc.vector.tensor_scalar(out=m[:], in0=pre[:], scalar1=0.0, scalar2=None, op0=OP.min)
    nc.scalar.activation(m[:], m[:], AF.Exp)
    ob = epi.tile([P, HC], F32, tag="ob", name=f"ob{b}")
    nc.vector.scalar_tensor_tensor(out=ob[:], in0=m[:], scalar=-1.0, in1=pre[:],
                                   op0=OP.add, op1=OP.max)
    nc.sync.dma_start(out=out_shard[b * P:(b + 1) * P, :], in_=ob[:])


# ----------------------------------------------------------------------------- driver

_CACHE = {}


def _run(x, edge_index, W, att_src, att_dst, bias, skip_W, trace=False):
    n_real = x.shape[0]
    plan = _plan(np.asarray(edge_index), n_real)
    bpc, n_pad, t_total = plan["bpc"], plan["n_pad"], plan["t_total"]
    w_np, has_bias = _weights(np.asarray(W, np.float32), np.asarray(att_src, np.float32),
                              np.asarray(att_dst, np.float32), np.asarray(bias, np.float32),
                              np.asarray(skip_W, np.float32))
    kdim = w_np.shape[0]

    key = (n_real, bpc, t_total, tuple(plan["tb"]), has_bias)
    if key not in _CACHE:
        _CACHE[key] = build_program(bpc, t_total, plan["tb"], plan["tstart"], has_bias)
    nc = _CACHE[key]

    x_np = np.asarray(x, np.float32)
    rows_c = bpc * P
    x_pad = np.zeros((n_pad, kdim), dtype=np.float32)
    x_pad[:n_real, :IN_DIM] = x_np
    if has_bias:
        x_pad[:n_real, IN_DIM] = 1.0

    in_maps = []
    for c in range(NCORES):
        xc = x_pad[c * rows_c:(c + 1) * rows_c]
        xT_blocks = np.ascontiguousarray(xc.reshape(bpc, P, kdim).transpose(0, 2, 1))
        in_maps.append(dict(
            xT_blocks=xT_blocks,
            w_all=w_np,
            src_T=np.ascontiguousarray(plan["src_T"][c]),
            rel_T=np.ascontiguousarray(plan["rel_T"][c]),
        ))

    res = run_bass_kernel_spmd(nc, in_maps, list(range(NCORES)), trace=trace)
    out = np.concatenate([res.results[c]["out_shard"] for c in range(NCORES)], axis=0)
    return out[:n_real], res


def kernel(x, edge_index, W, att_src, att_dst, bias, skip_W):
    out, _ = _run(x, edge_index, W, att_src, att_dst, bias, skip_W, trace=False)
    return out
